# revision 13
# baseline (speedup 1.0000x reference)
"""Multi-head self-attention (S=2048, B=2, D=1024, H=16) on 8 TRN2 NeuronCores.

Sharding: core c handles batch b = c//4 and head-quad g = c%4 (4 heads of 64).
Megatron-style: in_proj column-sliced, out_proj row-sliced; host sums the 8
partial outputs and adds out_proj bias.

Per-core dataflow (all matmuls fp32r):
  - host supplies x^T (D-major) activations and pre-transposed weight slices
  - qpT/kpT computed head-major (m on partitions, seq on free)
  - vp computed seq-major with an interleaved ones column per head (65-wide
    blocks) so the PV matmul also produces softmax row-sums on partition 64
  - scores^T per (head-pair, 512-query-chunk, key-tile) in a packed psum tile
    (128, 2, 512); exp on ACT reads the pair in one op
  - normalization via reciprocal + K=1 broadcast matmul + DVE multiply
  - out-projection on device from attn^T; bias + cross-core reduction on host
"""

import math
from contextlib import ExitStack

import numpy as np

S = 2048
B = 2
D = 1024
H = 16
DK = 64
HC = 4          # heads per core
M = HC * DK     # 256 head-dim columns per core
N_CORES = 8
KT = S // 128   # 16 key tiles
QQ = 4          # 512-wide query chunks
F32 = "float32"

_compiled = None


def _build_program():
    import concourse.bass as bass
    import concourse.tile as tile
    from concourse import mybir, bacc

    f32 = mybir.dt.float32
    f32r = mybir.dt.float32r
    EXP = mybir.ActivationFunctionType.Exp

    nc = bacc.Bacc("TRN2", target_bir_lowering=False, debug=False)

    xqT = nc.dram_tensor("xqT", [D, S], f32r, kind="ExternalInput").ap()
    xkT = nc.dram_tensor("xkT", [D, S], f32r, kind="ExternalInput").ap()
    xvT = nc.dram_tensor("xvT", [D, S], f32r, kind="ExternalInput").ap()
    wqT = nc.dram_tensor("wqT", [D, M], f32r, kind="ExternalInput").ap()
    wkT = nc.dram_tensor("wkT", [D, M], f32r, kind="ExternalInput").ap()
    wvT = nc.dram_tensor("wvT", [D, M], f32r, kind="ExternalInput").ap()
    bq = nc.dram_tensor("bq", [M], f32, kind="ExternalInput").ap()
    bk = nc.dram_tensor("bk", [M], f32, kind="ExternalInput").ap()
    bv = nc.dram_tensor("bv", [M], f32r, kind="ExternalInput").ap()
    woT = nc.dram_tensor("woT", [M, D], f32r, kind="ExternalInput").ap()
    ones_dr = nc.dram_tensor("ones", [1, 128], f32r, kind="ExternalInput").ap()
    vones_dr = nc.dram_tensor("vones", [128, KT, HC], f32r, kind="ExternalInput").ap()
    out = nc.dram_tensor("out", [S, D], f32, kind="ExternalOutput").ap()

    with tile.TileContext(nc) as tc, ExitStack() as ctx:
        const_pool = ctx.enter_context(tc.tile_pool(name="const", bufs=1))
        x_pool = ctx.enter_context(tc.tile_pool(name="x", bufs=8))
        e_pool = ctx.enter_context(tc.tile_pool(name="e", bufs=3))
        o_pool = ctx.enter_context(tc.tile_pool(name="o", bufs=2))
        r_pool = ctx.enter_context(tc.tile_pool(name="r", bufs=2))
        ps_a = ctx.enter_context(tc.tile_pool(name="ps_a", bufs=2, space="PSUM"))
        ps_b = ctx.enter_context(tc.tile_pool(name="ps_b", bufs=4, space="PSUM"))

        # ---- persistent SBUF tensors ----
        # weight slices as matmul lhsT, K-chunked: [p, kc, m]
        wq_sb = const_pool.tile([128, 8, M], f32r)
        wk_sb = const_pool.tile([128, 8, M], f32r)
        wv_sb = const_pool.tile([128, 8, M], f32r)
        for w_sb, w_dr in ((wq_sb, wqT), (wk_sb, wkT), (wv_sb, wvT)):
            nc.sync.dma_start(
                out=w_sb[:, :, :], in_=w_dr.rearrange("(kc p) m -> p kc m", p=128)
            )
        # out_proj rhs: [p, kc, j]
        wo_sb = const_pool.tile([128, 2, D], f32r)
        nc.sync.dma_start(
            out=wo_sb[:, :, :], in_=woT.rearrange("(kc p) j -> p kc j", p=128)
        )
        # per-partition biases for qpT/kpT: [p, mt]
        bq_sb = const_pool.tile([128, 2], f32)
        bk_sb = const_pool.tile([128, 2], f32)
        nc.sync.dma_start(out=bq_sb[:, :], in_=bq.rearrange("(mt p) -> p mt", p=128))
        nc.sync.dma_start(out=bk_sb[:, :], in_=bk.rearrange("(mt p) -> p mt", p=128))
        # bv as a K=1 matmul rhs row
        bv_sb = const_pool.tile([1, M], f32r)
        nc.sync.dma_start(out=bv_sb[:, :], in_=bv.rearrange("(a m) -> a m", a=1))
        ones_sb = const_pool.tile([1, 128], f32r)
        nc.sync.dma_start(out=ones_sb[:, :], in_=ones_dr[:, :])

        qpT = const_pool.tile([128, 2, S], f32r)   # [p, mt, s]
        kpT = const_pool.tile([128, 2, S], f32r)
        vp = const_pool.tile([128, KT, HC * 65], f32r)  # aug: 65-wide per head
        attnT = const_pool.tile([128, 2, S], f32r)

        # ones columns of the augmented V (once; head h at column h*65+64)
        nc.sync.dma_start(
            out=vp[:, :, :].rearrange("p kt (h c) -> p kt h c", c=65)[:, :, :, 64],
            in_=vones_dr[:, :, :],
        )

        # ---- projections ----
        # qpT/kpT: for each free-half load the 8 K-chunks of x^T once
        for x_dr, w_sb, b_sb, p_sb in (
            (xqT, wq_sb, bq_sb, qpT),
            (xkT, wk_sb, bk_sb, kpT),
        ):
            for half in range(2):
                fs = half * 1024
                chunks = []
                for kc in range(8):
                    xt = x_pool.tile([128, 1024], f32r, tag="xchunk")
                    nc.sync.dma_start(
                        out=xt[:, :], in_=x_dr[kc * 128:(kc + 1) * 128, fs:fs + 1024]
                    )
                    chunks.append(xt)
                for mt in range(2):
                    for nch in range(2):
                        ns = nch * 512
                        ps = ps_a.tile([128, 2, 512], f32, tag="ps_main")
                        for kc in range(8):
                            nc.tensor.matmul(
                                ps[:, 0, :],
                                w_sb[:, kc, mt * 128:(mt + 1) * 128],
                                chunks[kc][:, ns:ns + 512],
                                start=(kc == 0),
                                stop=(kc == 7),
                            )
                        nc.vector.tensor_scalar_add(
                            out=p_sb[:, mt, fs + ns:fs + ns + 512],
                            in0=ps[:, 0, :],
                            scalar1=b_sb[:, mt:mt + 1],
                        )

        # vp (seq-major, augmented layout), consumes x_v^T by free-half
        for half in range(2):
            fs = half * 1024
            chunks = []
            for kc in range(8):
                xt = x_pool.tile([128, 1024], f32r, tag="xchunk")
                nc.sync.dma_start(
                    out=xt[:, :], in_=xvT[kc * 128:(kc + 1) * 128, fs:fs + 1024]
                )
                chunks.append(xt)
            for st in range(8):
                kt = half * 8 + st
                ps = ps_a.tile([128, 2, 512], f32, tag="ps_main")
                for kc in range(8):
                    nc.tensor.matmul(
                        ps[:, 0, 0:M],
                        chunks[kc][:, st * 128:(st + 1) * 128],
                        wv_sb[:, kc, :],
                        start=(kc == 0),
                        stop=False,
                    )
                # bias via K=1 ones-row matmul
                nc.tensor.matmul(
                    ps[:, 0, 0:M],
                    ones_sb[0:1, 0:128],
                    bv_sb[0:1, :],
                    start=False,
                    stop=True,
                )
                nc.vector.tensor_copy(
                    out=vp[:, kt, :].rearrange("p (h c) -> p h c", c=65)[:, :, 0:64],
                    in_=ps[:, 0, 0:M].rearrange("p (h c) -> p h c", c=64),
                )

        # ---- attention + out-projection ----
        for qq in range(QQ):
            qs = qq * 512
            for pair in range(2):
                u_tiles = []
                for h in (2 * pair, 2 * pair + 1):
                    u_tiles.append(
                        ps_b.tile([65, 512], f32, tag="ps_small", name=f"u_{qq}_{h}")
                    )
                for kt in range(KT):
                    ks = kt * 128
                    sc = ps_a.tile([128, 2, 512], f32, tag="ps_main")
                    for hh in range(2):
                        h = 2 * pair + hh
                        po = hh * 64
                        nc.tensor.matmul(
                            sc[:, hh, :],
                            kpT[po:po + 64, pair, ks:ks + 128],
                            qpT[po:po + 64, pair, qs:qs + 512],
                            start=True,
                            stop=True,
                        )
                    et = e_pool.tile([128, 2, 512], f32r)
                    nc.scalar.activation(out=et[:, :, :], in_=sc[:, :, :], func=EXP)
                    for hh in range(2):
                        h = 2 * pair + hh
                        nc.tensor.matmul(
                            u_tiles[hh][0:65, :],
                            vp[:, kt, h * 65:(h + 1) * 65],
                            et[:, hh, :],
                            start=(kt == 0),
                            stop=(kt == KT - 1),
                        )
                # normalize: attnT = U / rowsum
                for hh in range(2):
                    h = 2 * pair + hh
                    u = u_tiles[hh]
                    rec = r_pool.tile([1, 512], f32r)
                    with nc.allow_low_precision(reason="softmax denom fits f32r"):
                        nc.vector.reciprocal(out=rec[:, :], in_=u[64:65, :])
                    rb = ps_b.tile([64, 512], f32, tag="ps_small")
                    nc.tensor.matmul(
                        rb[0:64, :],
                        ones_sb[0:1, 0:64],
                        rec[0:1, :],
                        start=True,
                        stop=True,
                    )
                    rbs = r_pool.tile([64, 512], f32, tag="rbs")
                    nc.vector.tensor_copy(out=rbs[:, :], in_=rb[0:64, :])
                    nc.vector.tensor_tensor(
                        out=attnT[hh * 64:hh * 64 + 64, pair, qs:qs + 512],
                        in0=u[0:64, :],
                        in1=rbs[0:64, :],
                        op=mybir.AluOpType.mult,
                    )
            # out-projection for the 4 finished s-tiles of this query chunk
            for st in range(4):
                sg = qq * 4 + st
                ot = o_pool.tile([128, D], f32)
                for nch in range(2):
                    ns = nch * 512
                    po = ps_b.tile([128, 512], f32, tag="ps_small")
                    for kc in range(2):
                        nc.tensor.matmul(
                            po[:, :],
                            attnT[:, kc, sg * 128:(sg + 1) * 128],
                            wo_sb[:, kc, ns:ns + 512],
                            start=(kc == 0),
                            stop=(kc == 1),
                        )
                    nc.vector.tensor_copy(out=ot[:, ns:ns + 512], in_=po[:, :])
                nc.sync.dma_start(out=out[sg * 128:(sg + 1) * 128, :], in_=ot[:, :])

    nc.compile()
    return nc


def _get_compiled():
    global _compiled
    if _compiled is None:
        _compiled = _build_program()
    return _compiled


def _make_in_maps(q, k, v, in_proj_w, in_proj_b, out_proj_w):
    # host-side shard prep
    xT = {}
    for b in range(B):
        xT[b] = (
            np.ascontiguousarray(q[:, b, :].T),
            np.ascontiguousarray(k[:, b, :].T),
            np.ascontiguousarray(v[:, b, :].T),
        )
    scale = 1.0 / math.sqrt(DK)
    in_maps = []
    for c in range(N_CORES):
        b, g = divmod(c, HC)
        cols = slice(g * M, (g + 1) * M)
        in_maps.append({
            "xqT": xT[b][0],
            "xkT": xT[b][1],
            "xvT": xT[b][2],
            "wqT": np.ascontiguousarray((in_proj_w[0 * D:1 * D][cols] * scale).T),
            "wkT": np.ascontiguousarray(in_proj_w[1 * D:2 * D][cols].T),
            "wvT": np.ascontiguousarray(in_proj_w[2 * D:3 * D][cols].T),
            "bq": np.ascontiguousarray(in_proj_b[0 * D:1 * D][cols] * scale),
            "bk": np.ascontiguousarray(in_proj_b[1 * D:2 * D][cols]),
            "bv": np.ascontiguousarray(in_proj_b[2 * D:3 * D][cols]),
            "woT": np.ascontiguousarray(out_proj_w[:, g * M:(g + 1) * M].T),
            "ones": np.ones((1, 128), dtype=np.float32),
            "vones": np.ones((128, KT, HC), dtype=np.float32),
        })
    return in_maps


def kernel(q, k, v, in_proj_w, in_proj_b, out_proj_w, out_proj_b):
    from concourse.bass_utils import run_bass_kernel_spmd

    q = np.asarray(q, dtype=np.float32)
    k = np.asarray(k, dtype=np.float32)
    v = np.asarray(v, dtype=np.float32)
    in_proj_w = np.asarray(in_proj_w, dtype=np.float32)
    in_proj_b = np.asarray(in_proj_b, dtype=np.float32)
    out_proj_w = np.asarray(out_proj_w, dtype=np.float32)
    out_proj_b = np.asarray(out_proj_b, dtype=np.float32)

    nc = _get_compiled()
    in_maps = _make_in_maps(q, k, v, in_proj_w, in_proj_b, out_proj_w)

    res = run_bass_kernel_spmd(nc, in_maps, core_ids=list(range(N_CORES)))

    out = np.broadcast_to(out_proj_b.astype(np.float32), (S, B, D)).copy()
    for c in range(N_CORES):
        out[:, c // HC, :] += res.results[c]["out"]
    return out


# revision 16
# speedup vs baseline: 1.3820x; 1.3820x over previous
"""Multi-head self-attention (S=2048, B=2, D=1024, H=16) on 8 TRN2 NeuronCores.

Sharding: core c handles batch b = c//4 and head-quad g = c%4 (4 heads of 64).
Megatron-style: in_proj column-sliced, out_proj row-sliced; host sums the 8
partial outputs and adds out_proj bias.

Per-core dataflow (matmul inputs bf16, accumulation fp32):
  - host supplies x^T (D-major) activations and pre-transposed weight slices
  - qpT/kpT computed head-major (m on partitions, seq on free)
  - vp computed seq-major with an interleaved ones column per head (65-wide
    blocks) so the PV matmul also produces softmax row-sums on partition 64
  - scores^T per (head-pair, 512-query-chunk, key-tile) in a packed psum tile
    (128, 2, 512); exp on ACT reads the pair in one op
  - normalization: K=1 matmul broadcasts the row-sums, DVE divides
  - out-projection on device from attn^T; bias + cross-core reduction on host
"""

import math
from contextlib import ExitStack

import numpy as np

S = 2048
B = 2
D = 1024
H = 16
DK = 64
HC = 4          # heads per core
M = HC * DK     # 256 head-dim columns per core
N_CORES = 8
KT = S // 128   # 16 key tiles
QQ = 4          # 512-wide query chunks

MM_DT = "bfloat16"   # dtype of matmul inputs ("bfloat16" or "float32r")

_compiled = None


def _build_program():
    import concourse.tile as tile
    from concourse import mybir, bacc

    f32 = mybir.dt.float32
    f32r = mybir.dt.float32r
    mdt = getattr(mybir.dt, MM_DT)
    EXP = mybir.ActivationFunctionType.Exp

    nc = bacc.Bacc("TRN2", target_bir_lowering=False, debug=False)

    xqT = nc.dram_tensor("xqT", [D, S], mdt, kind="ExternalInput").ap()
    xkT = nc.dram_tensor("xkT", [D, S], mdt, kind="ExternalInput").ap()
    xvT = nc.dram_tensor("xvT", [D, S], mdt, kind="ExternalInput").ap()
    wqT = nc.dram_tensor("wqT", [D, M], mdt, kind="ExternalInput").ap()
    wkT = nc.dram_tensor("wkT", [D, M], mdt, kind="ExternalInput").ap()
    wvT = nc.dram_tensor("wvT", [D, M], mdt, kind="ExternalInput").ap()
    bq = nc.dram_tensor("bq", [M], f32, kind="ExternalInput").ap()
    bk = nc.dram_tensor("bk", [M], f32, kind="ExternalInput").ap()
    bv = nc.dram_tensor("bv", [M], mdt, kind="ExternalInput").ap()
    woT = nc.dram_tensor("woT", [M, D], mdt, kind="ExternalInput").ap()
    ones_dr = nc.dram_tensor("ones", [1, 128], mdt, kind="ExternalInput").ap()
    ones32_dr = nc.dram_tensor("ones32", [1, 64], f32r, kind="ExternalInput").ap()
    vones_dr = nc.dram_tensor("vones", [128, KT, HC], mdt, kind="ExternalInput").ap()
    out = nc.dram_tensor("out", [S, D], f32, kind="ExternalOutput").ap()

    with tile.TileContext(nc) as tc, ExitStack() as ctx:
        const_pool = ctx.enter_context(tc.tile_pool(name="const", bufs=1))
        x_pool = ctx.enter_context(tc.tile_pool(name="x", bufs=8))
        e_pool = ctx.enter_context(tc.tile_pool(name="e", bufs=4))
        o_pool = ctx.enter_context(tc.tile_pool(name="o", bufs=2))
        r_pool = ctx.enter_context(tc.tile_pool(name="r", bufs=2))
        ps_a = ctx.enter_context(tc.tile_pool(name="ps_a", bufs=2, space="PSUM"))
        ps_b = ctx.enter_context(tc.tile_pool(name="ps_b", bufs=4, space="PSUM"))

        # ---- persistent SBUF tensors ----
        # weight slices as matmul lhsT, K-chunked: [p, kc, m]
        wq_sb = const_pool.tile([128, 8, M], mdt)
        wk_sb = const_pool.tile([128, 8, M], mdt)
        wv_sb = const_pool.tile([128, 8, M], mdt)
        for w_sb, w_dr in ((wq_sb, wqT), (wk_sb, wkT), (wv_sb, wvT)):
            nc.sync.dma_start(
                out=w_sb[:, :, :], in_=w_dr.rearrange("(kc p) m -> p kc m", p=128)
            )
        # out_proj rhs: [p, kc, j]
        wo_sb = const_pool.tile([128, 2, D], mdt)
        nc.sync.dma_start(
            out=wo_sb[:, :, :], in_=woT.rearrange("(kc p) j -> p kc j", p=128)
        )
        # per-partition biases for qpT/kpT: [p, mt]
        bq_sb = const_pool.tile([128, 2], f32)
        bk_sb = const_pool.tile([128, 2], f32)
        nc.sync.dma_start(out=bq_sb[:, :], in_=bq.rearrange("(mt p) -> p mt", p=128))
        nc.sync.dma_start(out=bk_sb[:, :], in_=bk.rearrange("(mt p) -> p mt", p=128))
        # bv as a K=1 matmul rhs row
        bv_sb = const_pool.tile([1, M], mdt)
        nc.sync.dma_start(out=bv_sb[:, :], in_=bv.rearrange("(a m) -> a m", a=1))
        ones_sb = const_pool.tile([1, 128], mdt)
        nc.sync.dma_start(out=ones_sb[:, :], in_=ones_dr[:, :])
        ones32_sb = const_pool.tile([1, 64], f32r)
        nc.sync.dma_start(out=ones32_sb[:, :], in_=ones32_dr[:, :])

        qpT = const_pool.tile([128, 2, S], mdt)   # [p, mt, s]
        kpT = const_pool.tile([128, 2, S], mdt)
        vp = const_pool.tile([128, KT, HC * 65], mdt)  # aug: 65-wide per head
        attnT = const_pool.tile([128, 2, S], mdt)

        # ones columns of the augmented V (once; head h at column h*65+64)
        nc.sync.dma_start(
            out=vp[:, :, :].rearrange("p kt (h c) -> p kt h c", c=65)[:, :, :, 64],
            in_=vones_dr[:, :, :],
        )

        # ---- projections ----
        # qpT/kpT: for each free-half load the 8 K-chunks of x^T once
        for x_dr, w_sb, b_sb, p_sb in (
            (xqT, wq_sb, bq_sb, qpT),
            (xkT, wk_sb, bk_sb, kpT),
        ):
            for half in range(2):
                fs = half * 1024
                chunks = []
                for kc in range(8):
                    xt = x_pool.tile([128, 1024], mdt, tag="xchunk")
                    nc.sync.dma_start(
                        out=xt[:, :], in_=x_dr[kc * 128:(kc + 1) * 128, fs:fs + 1024]
                    )
                    chunks.append(xt)
                for mt in range(2):
                    for nch in range(2):
                        ns = nch * 512
                        ps = ps_a.tile([128, 2, 512], f32, tag="ps_main")
                        for kc in range(8):
                            nc.tensor.matmul(
                                ps[:, 0, :],
                                w_sb[:, kc, mt * 128:(mt + 1) * 128],
                                chunks[kc][:, ns:ns + 512],
                                start=(kc == 0),
                                stop=(kc == 7),
                            )
                        nc.vector.tensor_scalar_add(
                            out=p_sb[:, mt, fs + ns:fs + ns + 512],
                            in0=ps[:, 0, :],
                            scalar1=b_sb[:, mt:mt + 1],
                        )

        # vp (seq-major, augmented layout), consumes x_v^T by free-half
        for half in range(2):
            fs = half * 1024
            chunks = []
            for kc in range(8):
                xt = x_pool.tile([128, 1024], mdt, tag="xchunk")
                nc.sync.dma_start(
                    out=xt[:, :], in_=xvT[kc * 128:(kc + 1) * 128, fs:fs + 1024]
                )
                chunks.append(xt)
            for st in range(8):
                kt = half * 8 + st
                ps = ps_a.tile([128, 2, 512], f32, tag="ps_main")
                for kc in range(8):
                    nc.tensor.matmul(
                        ps[:, 0, 0:M],
                        chunks[kc][:, st * 128:(st + 1) * 128],
                        wv_sb[:, kc, :],
                        start=(kc == 0),
                        stop=False,
                    )
                # bias via K=1 ones-row matmul
                nc.tensor.matmul(
                    ps[:, 0, 0:M],
                    ones_sb[0:1, 0:128],
                    bv_sb[0:1, :],
                    start=False,
                    stop=True,
                )
                nc.vector.tensor_copy(
                    out=vp[:, kt, :].rearrange("p (h c) -> p h c", c=65)[:, :, 0:64],
                    in_=ps[:, 0, 0:M].rearrange("p (h c) -> p h c", c=64),
                )

        # ---- attention + out-projection ----
        for qq in range(QQ):
            qs = qq * 512
            for pair in range(2):
                u_tiles = []
                for h in (2 * pair, 2 * pair + 1):
                    u_tiles.append(
                        ps_b.tile([65, 512], f32, tag="ps_small", name=f"u_{qq}_{h}")
                    )
                for kt in range(KT):
                    ks = kt * 128
                    sc = ps_a.tile([128, 2, 512], f32, tag="ps_main")
                    for hh in range(2):
                        po = hh * 64
                        nc.tensor.matmul(
                            sc[:, hh, :],
                            kpT[po:po + 64, pair, ks:ks + 128],
                            qpT[po:po + 64, pair, qs:qs + 512],
                            start=True,
                            stop=True,
                        )
                    et = e_pool.tile([128, 2, 512], mdt)
                    nc.scalar.activation(out=et[:, :, :], in_=sc[:, :, :], func=EXP)
                    for hh in range(2):
                        h = 2 * pair + hh
                        nc.tensor.matmul(
                            u_tiles[hh][0:65, :],
                            vp[:, kt, h * 65:(h + 1) * 65],
                            et[:, hh, :],
                            start=(kt == 0),
                            stop=(kt == KT - 1),
                        )
                # normalize: attnT = U / rowsum (broadcast sums via K=1 matmul)
                for hh in range(2):
                    u = u_tiles[hh]
                    rs = r_pool.tile([1, 512], f32r, tag="rs")
                    with nc.allow_low_precision(reason="softmax denom"):
                        nc.vector.tensor_copy(out=rs[:, :], in_=u[64:65, :])
                    rb = ps_b.tile([64, 512], f32, tag="ps_small")
                    nc.tensor.matmul(
                        rb[0:64, :],
                        ones32_sb[0:1, 0:64],
                        rs[0:1, :],
                        start=True,
                        stop=True,
                    )
                    rbs = r_pool.tile([64, 512], f32, tag="rbs")
                    nc.vector.reciprocal_approx_fast(out=rbs[:, :], in_=rb[0:64, :])
                    with nc.allow_low_precision(reason="softmax normalize"):
                        nc.vector.tensor_tensor(
                            out=attnT[hh * 64:hh * 64 + 64, pair, qs:qs + 512],
                            in0=u[0:64, :],
                            in1=rbs[0:64, :],
                            op=mybir.AluOpType.mult,
                        )
            # out-projection for the 4 finished s-tiles of this query chunk
            for st in range(4):
                sg = qq * 4 + st
                ot = o_pool.tile([128, D], f32)
                for nch in range(2):
                    ns = nch * 512
                    po = ps_b.tile([128, 512], f32, tag="ps_small")
                    for kc in range(2):
                        nc.tensor.matmul(
                            po[:, :],
                            attnT[:, kc, sg * 128:(sg + 1) * 128],
                            wo_sb[:, kc, ns:ns + 512],
                            start=(kc == 0),
                            stop=(kc == 1),
                        )
                    nc.vector.tensor_copy(out=ot[:, ns:ns + 512], in_=po[:, :])
                nc.sync.dma_start(out=out[sg * 128:(sg + 1) * 128, :], in_=ot[:, :])

    nc.compile()
    return nc


def _get_compiled():
    global _compiled
    if _compiled is None:
        _compiled = _build_program()
    return _compiled


def _make_in_maps(q, k, v, in_proj_w, in_proj_b, out_proj_w):
    import ml_dtypes

    mdt_np = np.dtype(ml_dtypes.bfloat16) if MM_DT == "bfloat16" else np.float32

    def cvt(a):
        return np.ascontiguousarray(a).astype(mdt_np)

    xT = {}
    for b in range(B):
        xT[b] = (
            cvt(q[:, b, :].T),
            cvt(k[:, b, :].T),
            cvt(v[:, b, :].T),
        )
    scale = 1.0 / math.sqrt(DK)
    in_maps = []
    for c in range(N_CORES):
        b, g = divmod(c, HC)
        cols = slice(g * M, (g + 1) * M)
        in_maps.append({
            "xqT": xT[b][0],
            "xkT": xT[b][1],
            "xvT": xT[b][2],
            "wqT": cvt((in_proj_w[0 * D:1 * D][cols] * scale).T),
            "wkT": cvt(in_proj_w[1 * D:2 * D][cols].T),
            "wvT": cvt(in_proj_w[2 * D:3 * D][cols].T),
            "bq": np.ascontiguousarray(in_proj_b[0 * D:1 * D][cols] * scale),
            "bk": np.ascontiguousarray(in_proj_b[1 * D:2 * D][cols]),
            "bv": cvt(in_proj_b[2 * D:3 * D][cols]),
            "woT": cvt(out_proj_w[:, g * M:(g + 1) * M].T),
            "ones": np.ones((1, 128), dtype=mdt_np),
            "ones32": np.ones((1, 64), dtype=np.float32),
            "vones": np.ones((128, KT, HC), dtype=mdt_np),
        })
    return in_maps


def kernel(q, k, v, in_proj_w, in_proj_b, out_proj_w, out_proj_b):
    from concourse.bass_utils import run_bass_kernel_spmd

    q = np.asarray(q, dtype=np.float32)
    k = np.asarray(k, dtype=np.float32)
    v = np.asarray(v, dtype=np.float32)
    in_proj_w = np.asarray(in_proj_w, dtype=np.float32)
    in_proj_b = np.asarray(in_proj_b, dtype=np.float32)
    out_proj_w = np.asarray(out_proj_w, dtype=np.float32)
    out_proj_b = np.asarray(out_proj_b, dtype=np.float32)

    nc = _get_compiled()
    in_maps = _make_in_maps(q, k, v, in_proj_w, in_proj_b, out_proj_w)

    res = run_bass_kernel_spmd(nc, in_maps, core_ids=list(range(N_CORES)))

    out = np.broadcast_to(out_proj_b.astype(np.float32), (S, B, D)).copy()
    for c in range(N_CORES):
        out[:, c // HC, :] += res.results[c]["out"]
    return out
